# revision 64
# baseline (speedup 1.0000x reference)
"""AttnBlock kernel for Trainium2, 8 NeuronCores, data-parallel over batch.

Full-input contract: kernel(**inputs) takes the unsharded inputs
(x [8, 512, 2048] fp32 + groupnorm/conv params) and returns the full
[8, 512, 2048] fp32 output.  Each core processes one batch element end to
end (no collectives).

Design:
- Weights are pre-transposed and cast to fp8 (e4m3) on the host; all heavy
  matmuls run fp8 with DoubleRow perf mode (K=256/instruction, 0.5 cyc/row)
  accumulating in fp32 PSUM.  End-to-end relative error ~7e-4.
- GroupNorm stats: per-partition moments (bn_stats on DVE / fused
  Identity+Square accum on ACT), cross-partition group reduce via a tiny
  indicator matmul, normalization fused into one scale+shift pass.
- Attention is computed transposed (S^T = K^T Q per j-block) so softmax
  normalization is deferred: P~ = exp(S^T * C^-0.5) in fp8 (scores are
  bounded ~|1.7| so no max subtraction), denominator via a ones[128,128]
  matmul (broadcasts the row to all partitions for free), and the 1/denom
  scale rides the O = V P~^T PSUM->SBUF copy.
- The whole attention+projection pipeline is per-512-column i-chunk, so the
  ScalarE exp stream of chunk ic+1 overlaps PE/DVE work of chunk ic.
- v-bias folds through softmax (rows sum to 1) into the projection bias on
  the host: bo_eff = bo + wo @ bv.
"""

import sys

for _p in ("/opt/trn_rl_repo",):
    if _p not in sys.path:
        sys.path.append(_p)

import numpy as np
import ml_dtypes

import concourse.bass as bass
import concourse.bacc as bacc
import concourse.tile as tile
from concourse import mybir
from concourse import bass_utils

F32 = mybir.dt.float32
F8 = mybir.dt.float8e4

B, C, L = 8, 512, 2048
G = 8                      # groupnorm groups
EPS = 1e-6
P = 128                    # partitions
CT = C // P                # 4 channel tiles
NJ = L // P                # 16 j-blocks
SCALE = float(np.float32(C) ** -0.5)

_CACHE = {}


def build_program(reps=1):
    """reps>1 duplicates the whole compute body (same I/O) — used only to
    measure device execution time by differencing under async dispatch.

    All heavy matmuls run in fp8 (e4m3) with DoubleRow perf mode: operands
    are stored as [128, 2, n] pairs (contraction index c = pair*256 + i*128
    + p), giving K=256 per instruction at 0.5 cycles/row."""
    nc = bacc.Bacc("TRN2", target_bir_lowering=False, debug=False, num_devices=8)
    DR = mybir.MatmulPerfMode.DoubleRow
    NP = CT // 2   # channel pairs
    NJP = NJ // 2  # j-block pairs

    x_d = nc.dram_tensor("x", [C, L], F32, kind="ExternalInput").ap()
    wT_d = {m: nc.dram_tensor(f"w{m}T", [C, C], F8, kind="ExternalInput").ap()
            for m in "qkvo"}
    # packed per-channel consts [128, 22]: gamma|beta|bq|bk|bo (4 cols each)
    # then the 2-col group indicator (p//64 == g)
    pc_d = nc.dram_tensor("pc", [P, 22], F32, kind="ExternalInput").ap()
    indT_d = nc.dram_tensor("indT", [2, P], F32, kind="ExternalInput").ap()
    out_d = nc.dram_tensor("out", [C, L], F32, kind="ExternalOutput").ap()

    with tile.TileContext(nc) as tc:
        with (
            tc.tile_pool(name="weights", bufs=4) as pW,
            tc.tile_pool(name="x", bufs=CT) as pX,
            tc.tile_pool(name="ho", bufs=2 * NP) as pHO,
            tc.tile_pool(name="qk", bufs=2 * NP) as pQK,
            tc.tile_pool(name="vt", bufs=NJP) as pVT,
            tc.tile_pool(name="pt", bufs=NJP) as pPT,
            tc.tile_pool(name="r", bufs=1) as pR,
            tc.tile_pool(name="small", bufs=1) as pS,
            tc.tile_pool(name="fx", bufs=4) as pF,
            tc.tile_pool(name="ps", bufs=4, space="PSUM") as pp,
        ):
            # ---------- constants ----------
            # DMA order matters: the packed consts + x tiles feed the
            # groupnorm critical path, so they go down the (serialized) DMA
            # path first; weights are only needed once H exists.
            pc = pS.tile([P, 22], F32, tag="pc", name="pc")
            nc.sync.dma_start(out=pc, in_=pc_d)
            gamma_sb, beta_sb = pc[:, 0:4], pc[:, 4:8]
            bq_sb, bk_sb, bo_sb = pc[:, 8:12], pc[:, 12:16], pc[:, 16:20]
            ind_sb = pc[:, 20:22]
            indT_sb = pS.tile([2, P], F32, tag="indt", name="indT_sb")
            nc.sync.dma_start(out=indT_sb, in_=indT_d)
            w4 = {}

            def _load_w(m):
                t = pW.tile([P, CT, C], F8, tag="w", bufs=4, name=f"w4{m}")
                nc.sync.dma_start(
                    out=t, in_=wT_d[m].rearrange("(ct p) o -> p ct o", p=P))
                w4[m] = t

            X0 = []
            for t in range(CT):
                xt = pX.tile([P, L], F32, tag="x", name=f"pre_x{t}")
                for hf in range(2):
                    nc.sync.dma_start(
                        out=xt[:, hf * 1024:(hf + 1) * 1024],
                        in_=x_d[t * P:(t + 1) * P, hf * 1024:(hf + 1) * 1024])
                X0.append(xt)
            for m in "kqvo":
                _load_w(m)
            ones8 = pS.tile([P, 2, P], F8, tag="ones", name="ones8")
            nc.vector.memset(ones8, 1.0)
            zb = pS.tile([P, 1], F32, tag="zb", name="zb")
            nc.vector.memset(zb, 0.0)
            eps2 = pS.tile([2, 1], F32, tag="eps2", name="eps2")
            nc.vector.memset(eps2, EPS)
            # preload the sqrt act-table off the critical path (the set also
            # serves Identity/Square/Copy used by the stats passes)
            sqd = pS.tile([2, 1], F32, tag="sqd", name="sqd")
            nc.vector.memset(sqd, 1.0)
            nc.scalar.activation(out=sqd, in_=sqd,
                                 func=mybir.ActivationFunctionType.Sqrt,
                                 bias=eps2, scale=1.0)

            for rep in range(reps):
                # ---------- phase 1: x tiles (rep 0 prefetched), stats, H ----------
                if rep == 0:
                    X = X0
                else:
                    X = []
                    for t in range(CT):
                        xt = pX.tile([P, L], F32, tag="x", name=f"r{rep}_x{t}")
                        for hf in range(2):
                            nc.sync.dma_start(
                                out=xt[:, hf * 1024:(hf + 1) * 1024],
                                in_=x_d[t * P:(t + 1) * P, hf * 1024:(hf + 1) * 1024])
                        X.append(xt)
                Hp = [pHO.tile([P, 2, L], F8, tag="ho", name=f"r{rep}_hp{cp}")
                      for cp in range(NP)]
                for t in range(CT):
                    # stats split across engines so all four H tiles are
                    # ready early: tiles 1,3 on ACT (fused accum), 0,2 on DVE
                    act_path = t == 1
                    ht = Hp[t // 2][:, t % 2, :]
                    mv = pS.tile([P, 2], F32, tag="mv", bufs=2, name=f"r{rep}_mv{t}")
                    if act_path:
                        nc.scalar.activation(out=ht, in_=X[t],
                                             func=mybir.ActivationFunctionType.Identity,
                                             accum_out=mv[:, 0:1])
                        nc.scalar.activation(out=ht, in_=X[t],
                                             func=mybir.ActivationFunctionType.Square,
                                             accum_out=mv[:, 1:2])
                        norm = 1.0 / (64 * L)
                    else:
                        stats = pS.tile([P, 4, 6], F32, tag="bst", bufs=2,
                                        name=f"r{rep}_bst{t}")
                        xg = X[t].rearrange("p (s f) -> p s f", f=512)
                        for sg in range(4):
                            nc.vector.bn_stats(out=stats[:, sg, :], in_=xg[:, sg, :])
                        nc.vector.bn_aggr(out=mv, in_=stats)
                        # mv[:,1] := E[x^2] per partition = var + mean^2
                        m2 = pS.tile([P, 1], F32, tag="m2", bufs=2, name=f"r{rep}_m2{t}")
                        nc.vector.tensor_mul(m2, mv[:, 0:1], mv[:, 0:1])
                        nc.vector.tensor_add(mv[:, 1:2], mv[:, 1:2], m2)
                        norm = 1.0 / 64
                    # group-reduce over 64-partition halves: [2, 2] = ind.T @ mv
                    pst = pp.tile([2, 2], F32, tag="b1", name=f"r{rep}_pst{t}")
                    nc.tensor.matmul(pst, ind_sb, mv, start=True, stop=True)
                    gstat = pS.tile([2, 2], F32, tag="gstat", bufs=2,
                                    name=f"r{rep}_gstat{t}")
                    nc.scalar.mul(out=gstat, in_=pst, mul=norm)
                    # var_g = E[x^2]_g - mean_g^2 ; rstd = 1/sqrt(var+eps)
                    sq2 = pS.tile([2, 1], F32, tag="sq2", bufs=2, name=f"r{rep}_sq2{t}")
                    nc.vector.tensor_mul(sq2, gstat[:, 0:1], gstat[:, 0:1])
                    nc.vector.tensor_sub(gstat[:, 1:2], gstat[:, 1:2], sq2)
                    nc.scalar.activation(out=gstat[:, 1:2], in_=gstat[:, 1:2],
                                         func=mybir.ActivationFunctionType.Sqrt,
                                         bias=eps2, scale=1.0)
                    nc.vector.reciprocal(out=gstat[:, 1:2], in_=gstat[:, 1:2])
                    # broadcast [2,2] -> [128,2] via indT.T @ gstat
                    mrsp = pp.tile([P, 2], F32, tag="b1", name=f"r{rep}_mrsp{t}")
                    nc.tensor.matmul(mrsp, indT_sb, gstat, start=True, stop=True)
                    # scale_p = rstd*gamma ; shift_p = beta - mean*scale
                    scl = pS.tile([P, 1], F32, tag="scl", bufs=4, name=f"r{rep}_scl{t}")
                    nc.vector.tensor_mul(scl, mrsp[:, 1:2], gamma_sb[:, t:t + 1])
                    sht = pS.tile([P, 1], F32, tag="sht", bufs=4, name=f"r{rep}_sht{t}")
                    nc.vector.tensor_mul(sht, mrsp[:, 0:1], scl)
                    nc.vector.tensor_sub(sht, beta_sb[:, t:t + 1], sht)
                    if t < 2:
                        nc.gpsimd.tensor_scalar(out=ht, in0=X[t], scalar1=scl,
                                                scalar2=sht,
                                                op0=mybir.AluOpType.mult,
                                                op1=mybir.AluOpType.add)
                    else:
                        nc.vector.tensor_scalar(out=ht, in0=X[t], scalar1=scl,
                                                scalar2=sht,
                                                op0=mybir.AluOpType.mult,
                                                op1=mybir.AluOpType.add)

                # ---------- phase 2: Q, K (channels-major) and V^T ----------
                Qp = [pQK.tile([P, 2, L], F8, tag="qk", name=f"r{rep}_qp{cp}")
                      for cp in range(NP)]
                Kp = [pQK.tile([P, 2, L], F8, tag="qk", name=f"r{rep}_kp{cp}")
                      for cp in range(NP)]
                # K first (ot-major), then Q i-chunk-major so the attention
                # pipeline can start as soon as Q[:, ic=0] lands.
                for ot in range(CT):
                    acc = [pp.tile([P, 512], F32, tag="b1",
                                   name=f"r{rep}_kps{ot}_{lc}") for lc in range(4)]
                    for cp in range(NP):
                        lhs = w4["k"][:, cp * 2:(cp + 1) * 2, ot * P:(ot + 1) * P]
                        for lc in range(4):
                            nc.tensor.matmul(acc[lc], lhs,
                                             Hp[cp][:, :, lc * 512:(lc + 1) * 512],
                                             start=(cp == 0), stop=(cp == NP - 1),
                                             perf_mode=DR)
                    for lc in range(4):
                        d_ap = Kp[ot // 2][:, ot % 2, lc * 512:(lc + 1) * 512]
                        if lc % 2 == 0:
                            nc.vector.tensor_scalar_add(d_ap, acc[lc],
                                                        bk_sb[:, ot:ot + 1])
                        else:
                            nc.scalar.activation(
                                out=d_ap, in_=acc[lc],
                                func=mybir.ActivationFunctionType.Identity,
                                bias=bk_sb[:, ot:ot + 1])
                for lc in range(4):
                    acc = [pp.tile([P, 512], F32, tag="b1",
                                   name=f"r{rep}_qps{lc}_{ot}") for ot in range(CT)]
                    for cp in range(NP):
                        for ot in range(CT):
                            nc.tensor.matmul(
                                acc[ot],
                                w4["q"][:, cp * 2:(cp + 1) * 2, ot * P:(ot + 1) * P],
                                Hp[cp][:, :, lc * 512:(lc + 1) * 512],
                                start=(cp == 0), stop=(cp == NP - 1), perf_mode=DR)
                    for ot in range(CT):
                        nc.vector.tensor_scalar_add(
                            Qp[ot // 2][:, ot % 2, lc * 512:(lc + 1) * 512],
                            acc[ot], bq_sb[:, ot:ot + 1])
                VTp = [pVT.tile([P, 2, C], F8, tag="vt", name=f"r{rep}_vtp{jp}")
                       for jp in range(NJP)]
                for jb in range(NJ):
                    acc = pp.tile([P, 512], F32, tag="b1", name=f"r{rep}_vps{jb}")
                    for cp in range(NP):
                        nc.tensor.matmul(acc, Hp[cp][:, :, jb * P:(jb + 1) * P],
                                         w4["v"][:, cp * 2:(cp + 1) * 2, :],
                                         start=(cp == 0), stop=(cp == NP - 1),
                                         perf_mode=DR)
                    if jb % 2 == 0:
                        nc.vector.tensor_copy(VTp[jb // 2][:, jb % 2, :], acc)
                    else:
                        nc.scalar.activation(out=VTp[jb // 2][:, jb % 2, :], in_=acc,
                                             func=mybir.ActivationFunctionType.Identity,
                                             bias=zb)

                # ---------- attention + projection, pipelined per i-chunk ----
                # For each 512-wide query chunk: S^T -> exp (fp8 P^T) ->
                # denominator -> O = (V P^T) R -> proj -> (+bo +x) -> store.
                # The ACT exp stream of chunk ic+1 overlaps PE/DVE work of
                # chunk ic; P^T lives only as 8 per-chunk [128, 2, 512] tiles.
                Op = [pHO.tile([P, 2, L], F8, tag="ho", name=f"r{rep}_op{cp}")
                      for cp in range(NP)]
                R = pR.tile([P, L], F32, tag="r", name=f"r{rep}_rbc")
                for ic in range(4):
                    icsl = slice(ic * 512, (ic + 1) * 512)
                    PTi = [pPT.tile([P, 2, 512], F8, tag="pt", bufs=3 * NJP,
                                    name=f"r{rep}_pt{ic}_{jp}") for jp in range(NJP)]
                    dacc = pp.tile([P, 512], F32, tag="b1", name=f"r{rep}_dps{ic}")
                    for jp in range(NJP):
                        sps = pp.tile([P, 2, 512], F32, tag="b2", bufs=2,
                                      name=f"r{rep}_sps{ic}_{jp}")
                        for jb2 in range(2):
                            jb = jp * 2 + jb2
                            for cp in range(NP):
                                nc.tensor.matmul(sps[:, jb2, :],
                                                 Kp[cp][:, :, jb * P:(jb + 1) * P],
                                                 Qp[cp][:, :, icsl],
                                                 start=(cp == 0), stop=(cp == NP - 1),
                                                 perf_mode=DR)
                        nc.scalar.activation(out=PTi[jp], in_=sps,
                                             func=mybir.ActivationFunctionType.Exp,
                                             bias=zb, scale=SCALE)
                        # denominator rides the exp stream, off the critical path
                        nc.tensor.matmul(dacc, ones8, PTi[jp],
                                         start=(jp == 0), stop=(jp == NJP - 1),
                                         perf_mode=DR)
                    nc.vector.reciprocal(out=R[:, icsl], in_=dacc)
                    accs = [pp.tile([P, 512], F32, tag="b1", name=f"r{rep}_ops{ic}_{ct}")
                            for ct in range(CT)]
                    for jp in range(NJP):
                        for ct in range(CT):
                            nc.tensor.matmul(accs[ct],
                                             VTp[jp][:, :, ct * P:(ct + 1) * P],
                                             PTi[jp], start=(jp == 0),
                                             stop=(jp == NJP - 1), perf_mode=DR)
                    for ct in range(CT):
                        nc.vector.tensor_mul(Op[ct // 2][:, ct % 2, icsl],
                                             accs[ct], R[:, icsl])
                    uacc = [pp.tile([P, 512], F32, tag="b1", name=f"r{rep}_ups{ic}_{ot}")
                            for ot in range(CT)]
                    for cp in range(NP):
                        for ot in range(CT):
                            nc.tensor.matmul(uacc[ot],
                                             w4["o"][:, cp * 2:(cp + 1) * 2,
                                                     ot * P:(ot + 1) * P],
                                             Op[cp][:, :, icsl],
                                             start=(cp == 0), stop=(cp == NP - 1),
                                             perf_mode=DR)
                    for ot in range(CT):
                        fx = pF.tile([P, 512], F32, tag="fx", bufs=8,
                                     name=f"r{rep}_fx{ot}_{ic}")
                        # (proj + bo) + x in one DVE op
                        nc.vector.scalar_tensor_tensor(
                            out=fx, in0=uacc[ot], scalar=bo_sb[:, ot:ot + 1],
                            in1=X[ot][:, icsl],
                            op0=mybir.AluOpType.add, op1=mybir.AluOpType.add)
                        nc.sync.dma_start(out=out_d[ot * P:(ot + 1) * P, icsl],
                                          in_=fx)
    nc.compile()
    return nc


def _prep_core_inputs(x_b, consts):
    m = {"x": np.ascontiguousarray(x_b)}
    m.update(consts)
    return m


def _host_consts(gamma, beta, wq, bq, wk, bk, wv, bv, wo, bo):
    bf = ml_dtypes.bfloat16
    pack = lambda v: np.asarray(v, np.float32).reshape(CT, P).T
    ind = np.zeros((P, 2), np.float32)
    ind[:64, 0] = 1.0
    ind[64:, 1] = 1.0
    # v-bias folds through the attention average (softmax rows sum to 1):
    # o = V_blk @ softmax + bv  =>  proj out shifts by wo @ bv, a constant.
    bo_eff = np.asarray(bo, np.float64) + (
        np.asarray(wo, np.float64) @ np.asarray(bv, np.float64))
    pc = np.concatenate([pack(gamma), pack(beta), pack(bq), pack(bk),
                         pack(bo_eff.astype(np.float32)), ind], axis=1)
    f8 = ml_dtypes.float8_e4m3
    return {
        "wqT": np.ascontiguousarray(np.asarray(wq).T.astype(f8)),
        "wkT": np.ascontiguousarray(np.asarray(wk).T.astype(f8)),
        "wvT": np.ascontiguousarray(np.asarray(wv).T.astype(f8)),
        "woT": np.ascontiguousarray(np.asarray(wo).T.astype(f8)),
        "pc": np.ascontiguousarray(pc),
        "indT": np.ascontiguousarray(ind.T),
    }


def kernel(x, gamma, beta, wq, bq, wk, bk, wv, bv, wo, bo):
    if ("nc", 1) not in _CACHE:
        _CACHE[("nc", 1)] = build_program()
    nc = _CACHE[("nc", 1)]
    x = np.asarray(x, np.float32)
    consts = _host_consts(gamma, beta, wq, bq, wk, bk, wv, bv, wo, bo)
    in_maps = [_prep_core_inputs(x[b], consts) for b in range(B)]
    res = bass_utils.run_bass_kernel_spmd(nc, in_maps, list(range(B)))
    return np.stack([res.results[b]["out"] for b in range(B)]).astype(np.float32)


# ---------------------------------------------------------------------------
# Dev-only benchmark helper: replicate bass2jax.run_bass_via_pjrt's sharded
# executable, cache it, and time repeated dispatches with device-resident
# inputs (transfer excluded).
# ---------------------------------------------------------------------------
def _make_runner(reps=1, n_cores=B):
    import jax
    from jax.experimental.shard_map import shard_map
    from jax.sharding import Mesh, PartitionSpec
    from concourse import bass2jax
    from concourse.bass2jax import _bass_exec_p, install_neuronx_cc_hook
    from concourse import mybir as mb

    key = ("nc", reps)
    if key not in _CACHE:
        _CACHE[key] = build_program(reps=reps)
    nc = _CACHE[key]
    install_neuronx_cc_hook()

    partition_name = nc.partition_id_tensor.name if nc.partition_id_tensor else None
    in_names, out_names, out_avals = [], [], []
    for alloc in nc.m.functions[0].allocations:
        if not isinstance(alloc, mb.MemoryLocationSet):
            continue
        name = alloc.memorylocations[0].name
        if alloc.kind == "ExternalInput":
            if name != partition_name:
                in_names.append(name)
        elif alloc.kind == "ExternalOutput":
            out_names.append(name)
            out_avals.append(jax.core.ShapedArray(tuple(alloc.tensor_shape),
                                                  mb.dt.np(alloc.dtype)))
    n_params = len(in_names)
    all_names = in_names + out_names
    if partition_name is not None:
        all_names = all_names + [partition_name]

    def _body(*args):
        operands = list(args)
        if partition_name is not None:
            operands.append(bass2jax.partition_id_tensor())
        outs = _bass_exec_p.bind(
            *operands, out_avals=tuple(out_avals), in_names=tuple(all_names),
            out_names=tuple(out_names), lowering_input_output_aliases=(),
            sim_require_finite=True, sim_require_nnan=True, nc=nc)
        return tuple(outs)

    devices = jax.devices()[:n_cores]
    mesh = Mesh(np.asarray(devices), ("core",))
    n_outs = len(out_names)
    sharded = jax.jit(
        shard_map(_body, mesh=mesh,
                  in_specs=(PartitionSpec("core"),) * (n_params + n_outs),
                  out_specs=(PartitionSpec("core"),) * n_outs),
        donate_argnums=tuple(range(n_params, n_params + n_outs)),
        keep_unused=True)
    return sharded, in_names, out_names, out_avals, mesh


def bench(inp, reps_hi=9, iters=60, n_cores=1):
    """Estimate per-body device exec time.

    Sync-dispatch a reps_hi-times duplicated body and the 1x body
    interleaved, difference robust percentiles of the per-call wall times.
    Per-call dispatch overhead through the axon relay (~70-80 ms) cancels in
    the difference; the reps_hi-1 extra bodies provide the signal."""
    import time
    import jax
    import jax.numpy as jnp

    x = np.asarray(inp["x"], np.float32)
    consts = _host_consts(inp["gamma"], inp["beta"], inp["wq"], inp["bq"],
                          inp["wk"], inp["bk"], inp["wv"], inp["bv"],
                          inp["wo"], inp["bo"])
    m0 = _prep_core_inputs(x[0], consts)

    runners = {}
    for reps in (1, reps_hi):
        sharded, in_names, out_names, out_avals, mesh = _make_runner(
            reps=reps, n_cores=n_cores)
        dev_in = [jax.device_put(np.asarray(m0[n])) for n in in_names]

        def zeros(avals=tuple(out_avals)):
            return [jnp.zeros(av.shape, av.dtype) for av in avals]

        outs = sharded(*dev_in, *zeros())
        jax.block_until_ready(outs)
        runners[reps] = (sharded, dev_in, zeros)

    pairs = []
    order = [1, reps_hi]
    for _ in range(iters):
        order = order[::-1]
        vals = {}
        for reps in order:
            sharded, dev_in, zeros = runners[reps]
            z = zeros()
            jax.block_until_ready(z)
            t0 = time.perf_counter()
            outs = sharded(*dev_in, *z)
            jax.block_until_ready(outs)
            vals[reps] = time.perf_counter() - t0
        pairs.append((vals[reps_hi] - vals[1]) / (reps_hi - 1) * 1e9)
    a = np.sort(np.array(pairs))
    k = max(1, len(a) // 5)
    return float(np.mean(a[k:-k]))  # 20-80% trimmed mean of paired deltas
